# revision 36
# baseline (speedup 1.0000x reference)
"""Trainium2 Bass kernel for the attention layer:

    f = wf@x+bf; g = wg@x+bg; h = wh@x+bh            (1x1 convs, Ci=32)
    attn = softmax(f^T g, axis=-1)                   (per batch, N=4096)
    out = (wv @ (h @ attn^T) + bv) * gamma + x

Sharding: 8 cores = 4 batches x 2 query-halves (2048 queries each).
Each core receives the full (256, 4096) batch slice with its query half
permuted to the front, so the SPMD program uses fixed offsets.

v2 dataflow (ACT-exp bound, everything else hides behind it):
  - all matmul operands bf16 (PSUM accumulate stays fp32); fp32 kept
    only for the residual add.
  - logits: key chunks of 128 in groups of 2 PSUM banks; 4-way
    strip-replicated f/g so consecutive chunk matmuls row-pack into
    different PE bands.  ACT exp reads the 2-bank group in ONE call
    (1024 elems/lane) -> bf16 eT in SBUF.
  - x0 accumulation col-packed 2-way: even key chunks at tile_position
    (0,0) (psum rows 0-32), odd at (0,64) (rows 64-96); each half
    carries its own ones-column for the softmax denominator.  One
    interleaved accumulation chain per qchunk bank (start at kc==0,
    stop at kc==31).
  - cross-qchunk software pipeline: the PE stream interleaves next
    qchunk's logits with current qchunk's x0 (x0 lags the exps by a
    few groups, bounded by the eT pool), so ACT never starves at
    qchunk boundaries and there are no serial per-qchunk tails.
  - tail per qchunk: dB moved to partition 0 (gpsimd), d=dA+dB,
    reciprocal_approx_fast, PE outer-product broadcast of 1/d to 97
    partitions, normalize (bf16), project with stacked wv (K=97, rows
    33-63 zero; bias rows at 0 and 64 exploit dA/d + dB/d = 1),
    fp32 residual add, DMA out.
"""

import os
import numpy as np
import ml_dtypes

import concourse.bass as bass
import concourse.mybir as mybir
import concourse.tile as tile
from concourse import bacc
from concourse.bass import ts
from concourse.bass_utils import run_bass_kernel_spmd

F32 = mybir.dt.float32
F32R = mybir.dt.float32r
BF16 = mybir.dt.bfloat16
EXP = mybir.ActivationFunctionType.Exp

B, C, W, H = 4, 256, 64, 64
N = W * H            # 4096 keys/queries per batch
CI = 32              # inner channels
NCORES = 8
NQ = N // 2          # queries per core
QC = 512             # query chunk = one fp32 PSUM bank
NQC = NQ // QC       # 4 query chunks per core
KC = 128             # key chunk = partition dim
NKC = N // KC        # 32 key chunks
GRP = 2              # key chunks per ACT exp group (PSUM banks per call)
NGRP = NKC // GRP    # 16 groups per qchunk
NWARM = 24           # dummy matmuls to warm the PE clock gate

TRACE = False
LAST_EXEC_NS = None

_cached_nc = None


def _build():
    nc = bacc.Bacc(
        "TRN2", target_bir_lowering=False, debug=False, num_devices=NCORES
    )
    xbf_d = nc.dram_tensor("xbf", (C, N), BF16, kind="ExternalInput").ap()
    xbfh_d = nc.dram_tensor("xbfh", (128, 1024), BF16, kind="ExternalInput").ap()
    cb_d = nc.dram_tensor("cb", (128, 832), BF16, kind="ExternalInput").ap()
    cf_d = nc.dram_tensor("cf", (128, 2), F32, kind="ExternalInput").ap()
    out_d = nc.dram_tensor("out", (C, NQ), F32, kind="ExternalOutput").ap()

    xbfr = xbf_d.rearrange("(cc p) n -> p cc n", p=128)
    outr = out_d.rearrange("(oc p) n -> p oc n", p=128)

    with tile.TileContext(nc) as tc:
        with (
            tc.tile_pool(name="consts", bufs=1) as consts,
            tc.tile_pool(name="data", bufs=1) as data,
            tc.tile_pool(name="eTp", bufs=8) as eTp,
            tc.tile_pool(name="smallp", bufs=2) as smallp,
            tc.tile_pool(name="outp", bufs=3) as outp,
            tc.tile_pool(name="pl", bufs=2, space="PSUM") as pl,
            tc.tile_pool(name="pp", bufs=2, space="PSUM") as pp,
            tc.tile_pool(name="px0", bufs=2, space="PSUM") as px0,
        ):
            # ---- PE + ACT warm-up (overlaps the input DMAs) ----
            scratch = consts.tile([128, QC], F32)
            nc.vector.memset(scratch, 0.0)
            wps = pp.tile([128, QC], F32, tag="pp", name="warm")
            for i in range(NWARM):
                # small N so the warm-ups don't delay the real stream
                nc.tensor.matmul(
                    wps[:, 0:128], lhsT=scratch[:, 0:128],
                    rhs=scratch[:, 0:128],
                    start=True, stop=True, skip_group_check=True,
                )
            scratchB = consts.tile([128, 64], BF16)
            nc.vector.memset(scratchB, 0.0)
            scratch2 = consts.tile([1, 8], F32)
            nc.scalar.activation(out=scratch2, in_=scratch[0:1, 0:8], func=EXP)

            # ---- x (bf16 everywhere, also used for the residual) ----
            # dma_start triggers cost ~640 ns SERIAL on the Sync engine:
            # keep the transfer count minimal.  One flat head transfer
            # (cols 0-511, partition-contiguous host layout) unblocks
            # f0/g0; the rest arrives in 4 column stripes.
            xbf_sb = data.tile([128, 2, N], BF16)
            xbfh_r = xbfh_d.rearrange("p (cc n) -> p cc n", cc=2)
            cb_sb = consts.tile([128, 832], BF16)
            # the DMA queues process ~one descriptor per 40 ns, so the
            # startup-critical transfers are split into partition halves
            # (parallel queues) interleaved for earliest completion
            for h in range(2):
                nc.sync.dma_start(
                    out=cb_sb[64 * h : 64 * h + 64, :],
                    in_=cb_d[64 * h : 64 * h + 64, :],
                )
                nc.sync.dma_start(
                    out=xbf_sb[64 * h : 64 * h + 64, :, 0:512],
                    in_=xbfh_r[64 * h : 64 * h + 64],
                )
            for h in range(2):
                nc.sync.dma_start(
                    out=xbf_sb[64 * h : 64 * h + 64, :, 512:1024],
                    in_=xbfr[64 * h : 64 * h + 64, :, 512:1024],
                )
            cf_sb = consts.tile([128, 2], F32)
            nc.sync.dma_start(out=cf_sb, in_=cf_d)
            bf_sb = cf_sb[:, 0:1]
            bg_sb = cf_sb[:, 1:2]
            for h in range(2):
                nc.sync.dma_start(
                    out=xbf_sb[64 * h : 64 * h + 64, :, 1024:2048],
                    in_=xbfr[64 * h : 64 * h + 64, :, 1024:2048],
                )
            for c0, c1 in ((2048, 3072), (3072, 4096)):
                nc.sync.dma_start(
                    out=xbf_sb[:, :, c0:c1], in_=xbfr[:, :, c0:c1]
                )
            # ones at rows 0 and 64: the denominator-extractor stationary
            dones = consts.tile([97, 1], BF16)
            nc.vector.memset(dones, 0.0)
            nc.vector.memset(dones[0:1, :], 1.0)
            nc.vector.memset(dones[64:65, :], 1.0)

            f_sb = data.tile([128, NQ], BF16)
            g_sb = data.tile([128, N], BF16)
            hT_sb = data.tile([128, NKC, CI + 1], BF16)
            nc.vector.memset(hT_sb[:, :, 0:1], 1.0)

            # ---- pre-phase emitters (short-lived psum slots, pp pool) ----
            def emit_f(j):
                ps = pp.tile([128, QC], F32, tag="pp", name=f"psf{j}")
                for cc in range(2):
                    nc.tensor.matmul(
                        ps, lhsT=cb_sb[:, ts(cc, 128)],
                        rhs=xbf_sb[:, cc, ts(j, QC)],
                        start=cc == 0, stop=cc == 1,
                    )
                nc.vector.tensor_scalar_add(f_sb[:, ts(j, QC)], ps, bf_sb)

            def emit_g(j):
                ps = pp.tile([128, QC], F32, tag="pp", name=f"psg{j}")
                for cc in range(2):
                    nc.tensor.matmul(
                        ps, lhsT=cb_sb[:, 256 + 128 * cc : 384 + 128 * cc],
                        rhs=xbf_sb[:, cc, ts(j, QC)],
                        start=cc == 0, stop=cc == 1,
                    )
                nc.vector.tensor_scalar_add(g_sb[:, ts(j, QC)], ps, bg_sb)

            def emit_hT(slot):
                # 16 key chunks' hT per [128, 512] psum slot, one big
                # strided copy into the ones-augmented hT_sb layout
                ps = pp.tile([128, 16, 32], F32, tag="pp", name=f"psh{slot}")
                for m in range(16):
                    kc = 16 * slot + m
                    for cc in range(2):
                        nc.tensor.matmul(
                            ps[:, m, :],
                            lhsT=xbf_sb[:, cc, ts(kc, KC)],
                            rhs=cb_sb[:, 512 + 32 * cc : 544 + 32 * cc],
                            start=cc == 0, stop=cc == 1,
                            skip_group_check=True,
                        )
                nc.vector.tensor_copy(
                    hT_sb[:, 16 * slot : 16 * slot + 16, 1 : CI + 1], ps
                )

            # ---- main-loop emitters ----
            eT_tiles = {}

            def emit_logits_pair(groups):
                # issue the 4 chunk matmuls of two exp groups back-to-back
                # (4 distinct 32-row strips) for 4-way PE row packing
                tiles = [
                    (qt, pl.tile([128, GRP, QC], F32, tag="lg",
                                 name=f"lg{qt[0]}_{qt[1]}"))
                    for qt in groups
                ]
                for (q, t), ps in tiles:
                    for i in range(GRP):
                        kc = GRP * t + i
                        s = kc % 4
                        nc.tensor.matmul(
                            ps[:, i, :],
                            lhsT=g_sb[32 * s : 32 * s + 32, ts(kc, KC)],
                            rhs=f_sb[32 * s : 32 * s + 32, ts(q, QC)],
                            start=True, stop=True,
                            tile_position=(32 * s, 0),
                        )
                for (q, t), ps in tiles:
                    eT = eTp.tile([128, GRP, QC], BF16, tag="eT")
                    nc.scalar.activation(out=eT, in_=ps, func=EXP)
                    eT_tiles[(q, t)] = eT

            x0_tiles = {}
            cur_x0 = [None]

            fill_i = [0]

            def emit_fillers(n=2):
                # tiny matmuls into a short-lived psum slot: keep the PE
                # activity monitor from re-throttling the clock during
                # ACT-bound micro-idles
                fill_i[0] += 1
                fps = pp.tile([128, QC], F32, tag="pp",
                              name=f"fill{fill_i[0]}")
                for _ in range(n):
                    nc.tensor.matmul(
                        fps[0:64, 0:64], lhsT=scratchB[:, 0:64],
                        rhs=scratchB[:, 0:64],
                        start=True, stop=True, skip_group_check=True,
                    )

            def emit_x0(q, t):
                if t == 0:
                    x0_tiles[q] = px0.tile([128, QC], F32, tag="x0",
                                           name=f"x0_{q}")
                    cur_x0[0] = x0_tiles[q]
                x0 = x0_tiles[q]
                eT = eT_tiles.pop((q, t))
                single = False   # col-packed chains for every qchunk
                for i in range(GRP):
                    kc = GRP * t + i
                    par = 0 if single else kc % 2
                    # each parity chain starts/stops its own partition
                    # range (pending-zero marking is per written partition)
                    nc.tensor.matmul(
                        x0[64 * par : 64 * par + CI + 1, :],
                        lhsT=hT_sb[:, kc, :],
                        rhs=eT[:, i, :],
                        start=kc <= (0 if single else 1),
                        stop=kc >= NKC - (1 if single else 2),
                        tile_position=(0, 64 * par),
                        skip_group_check=True,
                    )

            tail_state = {}

            def emit_tail_pre(q):
                # copy x0 out of PSUM (frees the accumulator bank early);
                # unnormalized channels + partial denominators, bf16
                x0 = x0_tiles.pop(q)
                if cur_x0[0] is x0:
                    cur_x0[0] = None
                x0s = smallp.tile([97, QC], BF16, tag="x0s")
                nc.vector.tensor_copy(x0s, x0[0:97, :])
                tail_state[q] = x0s

            def emit_tail_mid(q):
                x0s = tail_state[q]
                # d = dA + dB via a 1-column matmul (ones at rows 0, 64)
                dps = pp.tile([128, QC], F32, tag="pp", name=f"d{q}")
                nc.tensor.matmul(
                    dps[0:1, :], lhsT=dones, rhs=x0s,
                    start=True, stop=True,
                )
                rcp = smallp.tile([1, QC], F32, tag="rcp")
                nc.vector.reciprocal_approx_fast(out=rcp, in_=dps[0:1, :])
                rcp_b = smallp.tile([97, QC], F32, tag="rcpb")
                nc.gpsimd.partition_broadcast(rcp_b, rcp, channels=97)
                x0a = smallp.tile([97, QC], BF16, tag="x0a")
                nc.vector.tensor_mul(x0a, x0s, rcp_b)
                tail_state[q] = x0a

            def emit_tail_post(q):
                x0a = tail_state.pop(q)
                rows = 97
                for oc in (0, 1):
                    vps = pp.tile([128, QC], F32, tag="pp", name=f"v{q}_{oc}")
                    nc.tensor.matmul(
                        vps, lhsT=cb_sb[0:rows, 576 + 128 * oc : 704 + 128 * oc],
                        rhs=x0a[0:rows, :],
                        start=True, stop=True,
                    )
                    ot = outp.tile([128, QC], F32)
                    nc.vector.tensor_add(ot, vps, xbf_sb[:, oc, ts(q, QC)])
                    if q == NQC - 1:
                        for h in range(2):
                            nc.sync.dma_start(
                                out=outr[64 * h : 64 * h + 64, oc, ts(q, QC)],
                                in_=ot[64 * h : 64 * h + 64, :],
                            )
                    else:
                        nc.sync.dma_start(out=outr[:, oc, ts(q, QC)], in_=ot)

            # ---- emission schedule ----
            # qchunk 0 carries the pre-phase (f/g/hT) in its logits slots
            pre = {
                0: [lambda: emit_f(0), lambda: emit_g(0)],
                2: [lambda: emit_g(1)],
                4: [lambda: emit_g(2)],
                5: [lambda: emit_hT(0)],
                6: [lambda: emit_g(3)],
                7: [lambda: emit_f(1)],
                8: [lambda: emit_g(4)],
                9: [lambda: emit_hT(1)],
                10: [lambda: emit_g(5)],
                11: [lambda: emit_f(2)],
                12: [lambda: emit_g(6)],
                13: [lambda: emit_f(3)],
                14: [lambda: emit_g(7)],
            }
            seq = [(q, t) for q in range(NQC) for t in range(NGRP)]
            lg_i = 0
            x0_i = 0
            slot = 0
            deferred = []   # (due_slot, fn) in due order
            while lg_i < len(seq) or x0_i < len(seq) or deferred:
                if lg_i < len(seq):
                    pair = seq[lg_i : lg_i + 2]
                    for q, t in pair:
                        if q == 0:
                            for fn in pre.get(t, []):
                                fn()
                    emit_logits_pair(pair)
                    lg_i += len(pair)
                # x0 lags the exps (last qchunk drains eagerly instead)
                lag_req = 3
                budget = 2 if lg_i < len(seq) else len(seq)
                if lg_i - x0_i > 5:
                    budget = 3
                while (
                    budget > 0
                    and x0_i < len(seq)
                    and (lg_i - x0_i >= lag_req or lg_i >= len(seq))
                    and seq[x0_i] in eT_tiles
                ):
                    qx, tx = seq[x0_i]
                    # x0 must trail the hT emission (PE FIFO order):
                    # hT slot 0 lands at lg group 5, slot 1 at group 9
                    if qx == 0 and tx < 8 and lg_i < 6:
                        break
                    if qx == 0 and tx >= 8 and lg_i < 10:
                        break
                    emit_x0(qx, tx)
                    if lg_i >= len(seq):
                        emit_fillers(1)
                    x0_i += 1
                    budget -= 1
                    if tx == NGRP - 1:
                        emit_tail_pre(qx)
                        deferred.append(
                            (slot + 2, lambda q=qx: emit_tail_mid(q)))
                        deferred.append(
                            (slot + 3, lambda q=qx: emit_tail_post(q)))
                emit_fillers(2)
                while deferred and (
                    deferred[0][0] <= slot or
                    (lg_i >= len(seq) and x0_i >= len(seq))
                ):
                    deferred.pop(0)[1]()
                slot += 1

    nc.compile()
    return nc


def kernel(x, wf, bf, wg, bg, wh, bh, wv, bv, gamma):
    global _cached_nc, LAST_EXEC_NS
    if _cached_nc is None:
        _cached_nc = _build()
    nc = _cached_nc

    x = np.asarray(x, dtype=np.float32)
    wf = np.asarray(wf, dtype=np.float32)
    bf = np.asarray(bf, dtype=np.float32)
    wg = np.asarray(wg, dtype=np.float32)
    bg = np.asarray(bg, dtype=np.float32)
    wh = np.asarray(wh, dtype=np.float32)
    bh = np.asarray(bh, dtype=np.float32)
    wv = np.asarray(wv, dtype=np.float32)
    bv = np.asarray(bv, dtype=np.float32)
    g0 = float(np.asarray(gamma, dtype=np.float32).reshape(-1)[0])

    xf = np.ascontiguousarray(x.reshape(B, C, N))
    # f/g weights replicated 4x along M so f/g land replicated on the
    # four 32-partition strips (enables row-packed logits matmuls).
    bft = ml_dtypes.bfloat16
    wfT = np.tile(wf.T, (1, 4)).astype(np.float32)        # (256, 128)
    wgT = np.tile(wg.T, (1, 4)).astype(np.float32)
    whT = wh.T.astype(np.float32)                         # (256, 32)
    # stacked wv for the K=97 projection: bias rows at 0 and 64 (the
    # normalized denominators sum to 1), wv.T at rows 1-32 and 65-96.
    bias = g0 * (bv + wv @ bh)
    wvT = np.zeros((97, C), np.float32)
    wvT[0, :] = bias
    wvT[64, :] = bias
    wvT[1 : CI + 1, :] = g0 * wv.T
    wvT[65 : 65 + CI, :] = g0 * wv.T
    # single bf16 const blob: per-partition [wfT(2x128) wgT(2x128)
    # whT(2x32) wvT(2x128)]
    cb = np.zeros((128, 832), np.float32)
    cb[:, 0:128] = wfT[0:128, :]
    cb[:, 128:256] = wfT[128:256, :]
    cb[:, 256:384] = wgT[0:128, :]
    cb[:, 384:512] = wgT[128:256, :]
    cb[:, 512:544] = whT[0:128, :]
    cb[:, 544:576] = whT[128:256, :]
    cb[0:97, 576:832] = wvT
    cb = np.ascontiguousarray(cb.astype(bft))
    cf = np.ascontiguousarray(
        np.stack([np.tile(bf, 4), np.tile(bg, 4)], axis=1)
    ).astype(np.float32)

    in_maps = []
    for core in range(NCORES):
        b, half = divmod(core, 2)
        xb = xf[b]
        if half:
            xb = np.ascontiguousarray(
                np.concatenate([xb[:, NQ:], xb[:, :NQ]], axis=1)
            )
        xbb = xb.astype(bft)
        xbh = np.ascontiguousarray(
            np.concatenate([xbb[0:128, 0:512], xbb[128:256, 0:512]], axis=1)
        )
        in_maps.append({"xbf": xbb, "xbfh": xbh, "cb": cb, "cf": cf})

    res = run_bass_kernel_spmd(
        nc, in_maps, list(range(NCORES)),
        trace=TRACE or bool(os.environ.get("BASS_KERNEL_TRACE")),
    )
    LAST_EXEC_NS = res.exec_time_ns

    out = np.empty((B, C, N), np.float32)
    for core in range(NCORES):
        b, half = divmod(core, 2)
        out[b][:, half * NQ : (half + 1) * NQ] = res.results[core]["out"]
    return out.reshape(B, C, W, H)


# revision 37
# speedup vs baseline: 1.0263x; 1.0263x over previous
"""Trainium2 Bass kernel for the attention layer:

    f = wf@x+bf; g = wg@x+bg; h = wh@x+bh            (1x1 convs, Ci=32)
    attn = softmax(f^T g, axis=-1)                   (per batch, N=4096)
    out = (wv @ (h @ attn^T) + bv) * gamma + x

Sharding: 8 cores = 4 batches x 2 query-halves (2048 queries each).
Each core receives the full (256, 4096) batch slice with its query half
permuted to the front, so the SPMD program uses fixed offsets.

v2 dataflow (ACT-exp bound, everything else hides behind it):
  - all matmul operands bf16 (PSUM accumulate stays fp32); fp32 kept
    only for the residual add.
  - logits: key chunks of 128 in groups of 2 PSUM banks; 4-way
    strip-replicated f/g so consecutive chunk matmuls row-pack into
    different PE bands.  ACT exp reads the 2-bank group in ONE call
    (1024 elems/lane) -> bf16 eT in SBUF.
  - x0 accumulation col-packed 2-way: even key chunks at tile_position
    (0,0) (psum rows 0-32), odd at (0,64) (rows 64-96); each half
    carries its own ones-column for the softmax denominator.  One
    interleaved accumulation chain per qchunk bank (start at kc==0,
    stop at kc==31).
  - cross-qchunk software pipeline: the PE stream interleaves next
    qchunk's logits with current qchunk's x0 (x0 lags the exps by a
    few groups, bounded by the eT pool), so ACT never starves at
    qchunk boundaries and there are no serial per-qchunk tails.
  - tail per qchunk: dB moved to partition 0 (gpsimd), d=dA+dB,
    reciprocal_approx_fast, PE outer-product broadcast of 1/d to 97
    partitions, normalize (bf16), project with stacked wv (K=97, rows
    33-63 zero; bias rows at 0 and 64 exploit dA/d + dB/d = 1),
    fp32 residual add, DMA out.
"""

import os
import numpy as np
import ml_dtypes

import concourse.bass as bass
import concourse.mybir as mybir
import concourse.tile as tile
from concourse import bacc
from concourse.bass import ts
from concourse.bass_utils import run_bass_kernel_spmd

F32 = mybir.dt.float32
F32R = mybir.dt.float32r
BF16 = mybir.dt.bfloat16
EXP = mybir.ActivationFunctionType.Exp

B, C, W, H = 4, 256, 64, 64
N = W * H            # 4096 keys/queries per batch
CI = 32              # inner channels
NCORES = 8
NQ = N // 2          # queries per core
QC = 512             # query chunk = one fp32 PSUM bank
NQC = NQ // QC       # 4 query chunks per core
KC = 128             # key chunk = partition dim
NKC = N // KC        # 32 key chunks
GRP = 2              # key chunks per ACT exp group (PSUM banks per call)
NGRP = NKC // GRP    # 16 groups per qchunk
NWARM = 24           # dummy matmuls to warm the PE clock gate

TRACE = False
LAST_EXEC_NS = None

_cached_nc = None


def _build():
    nc = bacc.Bacc(
        "TRN2", target_bir_lowering=False, debug=False, num_devices=NCORES
    )
    xbf_d = nc.dram_tensor("xbf", (C, N), BF16, kind="ExternalInput").ap()
    xbfh_d = nc.dram_tensor("xbfh", (128, 1024), BF16, kind="ExternalInput").ap()
    cb_d = nc.dram_tensor("cb", (128, 832), BF16, kind="ExternalInput").ap()
    cf_d = nc.dram_tensor("cf", (128, 2), F32, kind="ExternalInput").ap()
    out_d = nc.dram_tensor("out", (C, NQ), F32, kind="ExternalOutput").ap()

    xbfr = xbf_d.rearrange("(cc p) n -> p cc n", p=128)
    outr = out_d.rearrange("(oc p) n -> p oc n", p=128)

    with tile.TileContext(nc) as tc:
        with (
            tc.tile_pool(name="consts", bufs=1) as consts,
            tc.tile_pool(name="data", bufs=1) as data,
            tc.tile_pool(name="eTp", bufs=8) as eTp,
            tc.tile_pool(name="smallp", bufs=2) as smallp,
            tc.tile_pool(name="outp", bufs=3) as outp,
            tc.tile_pool(name="pl", bufs=2, space="PSUM") as pl,
            tc.tile_pool(name="pp", bufs=2, space="PSUM") as pp,
            tc.tile_pool(name="px0", bufs=2, space="PSUM") as px0,
        ):
            # ---- PE + ACT warm-up (overlaps the input DMAs) ----
            scratch = consts.tile([128, QC], F32)
            nc.vector.memset(scratch, 0.0)
            wps = pp.tile([128, QC], F32, tag="pp", name="warm")
            for i in range(NWARM):
                # small N so the warm-ups don't delay the real stream
                nc.tensor.matmul(
                    wps[:, 0:128], lhsT=scratch[:, 0:128],
                    rhs=scratch[:, 0:128],
                    start=True, stop=True, skip_group_check=True,
                )
            scratchB = consts.tile([128, 64], BF16)
            nc.vector.memset(scratchB, 0.0)
            scratch2 = consts.tile([1, 8], F32)
            nc.scalar.activation(out=scratch2, in_=scratch[0:1, 0:8], func=EXP)

            # ---- x (bf16 everywhere, also used for the residual) ----
            # dma_start triggers cost ~640 ns SERIAL on the Sync engine:
            # keep the transfer count minimal.  One flat head transfer
            # (cols 0-511, partition-contiguous host layout) unblocks
            # f0/g0; the rest arrives in 4 column stripes.
            xbf_sb = data.tile([128, 2, N], BF16)
            nc.sync.dma_start(
                out=xbf_sb[:, :, 0:512],
                in_=xbfh_d.rearrange("p (cc n) -> p cc n", cc=2),
            )
            # ---- constants: one bf16 blob + one f32 blob ----
            cb_sb = consts.tile([128, 832], BF16)
            nc.sync.dma_start(out=cb_sb, in_=cb_d)
            cf_sb = consts.tile([128, 2], F32)
            nc.sync.dma_start(out=cf_sb, in_=cf_d)
            bf_sb = cf_sb[:, 0:1]
            bg_sb = cf_sb[:, 1:2]
            # rest of xbf (columns 512-4095) in 4 stripes
            for c0, c1 in ((512, 1024), (1024, 2048), (2048, 3072),
                           (3072, 4096)):
                nc.sync.dma_start(
                    out=xbf_sb[:, :, c0:c1], in_=xbfr[:, :, c0:c1]
                )
            # ones at rows 0 and 64: the denominator-extractor stationary
            dones = consts.tile([97, 1], BF16)
            nc.vector.memset(dones, 0.0)
            nc.vector.memset(dones[0:1, :], 1.0)
            nc.vector.memset(dones[64:65, :], 1.0)

            f_sb = data.tile([128, NQ], BF16)
            g_sb = data.tile([128, N], BF16)
            hT_sb = data.tile([128, NKC, CI + 1], BF16)
            nc.vector.memset(hT_sb[:, :, 0:1], 1.0)

            # ---- pre-phase emitters (short-lived psum slots, pp pool) ----
            def emit_f(j):
                ps = pp.tile([128, QC], F32, tag="pp", name=f"psf{j}")
                for cc in range(2):
                    nc.tensor.matmul(
                        ps, lhsT=cb_sb[:, ts(cc, 128)],
                        rhs=xbf_sb[:, cc, ts(j, QC)],
                        start=cc == 0, stop=cc == 1,
                    )
                nc.vector.tensor_scalar_add(f_sb[:, ts(j, QC)], ps, bf_sb)

            def emit_g(j):
                ps = pp.tile([128, QC], F32, tag="pp", name=f"psg{j}")
                for cc in range(2):
                    nc.tensor.matmul(
                        ps, lhsT=cb_sb[:, 256 + 128 * cc : 384 + 128 * cc],
                        rhs=xbf_sb[:, cc, ts(j, QC)],
                        start=cc == 0, stop=cc == 1,
                    )
                nc.vector.tensor_scalar_add(g_sb[:, ts(j, QC)], ps, bg_sb)

            def emit_hT(slot):
                # 16 key chunks' hT per [128, 512] psum slot, one big
                # strided copy into the ones-augmented hT_sb layout
                ps = pp.tile([128, 16, 32], F32, tag="pp", name=f"psh{slot}")
                for m in range(16):
                    kc = 16 * slot + m
                    for cc in range(2):
                        nc.tensor.matmul(
                            ps[:, m, :],
                            lhsT=xbf_sb[:, cc, ts(kc, KC)],
                            rhs=cb_sb[:, 512 + 32 * cc : 544 + 32 * cc],
                            start=cc == 0, stop=cc == 1,
                            skip_group_check=True,
                        )
                nc.vector.tensor_copy(
                    hT_sb[:, 16 * slot : 16 * slot + 16, 1 : CI + 1], ps
                )

            # ---- main-loop emitters ----
            eT_tiles = {}

            def emit_logits_pair(groups):
                # issue the 4 chunk matmuls of two exp groups back-to-back
                # (4 distinct 32-row strips) for 4-way PE row packing
                tiles = [
                    (qt, pl.tile([128, GRP, QC], F32, tag="lg",
                                 name=f"lg{qt[0]}_{qt[1]}"))
                    for qt in groups
                ]
                for (q, t), ps in tiles:
                    for i in range(GRP):
                        kc = GRP * t + i
                        s = kc % 4
                        nc.tensor.matmul(
                            ps[:, i, :],
                            lhsT=g_sb[32 * s : 32 * s + 32, ts(kc, KC)],
                            rhs=f_sb[32 * s : 32 * s + 32, ts(q, QC)],
                            start=True, stop=True,
                            tile_position=(32 * s, 0),
                        )
                for (q, t), ps in tiles:
                    eT = eTp.tile([128, GRP, QC], BF16, tag="eT")
                    nc.scalar.activation(out=eT, in_=ps, func=EXP)
                    eT_tiles[(q, t)] = eT

            x0_tiles = {}
            cur_x0 = [None]

            fill_i = [0]

            def emit_fillers(n=2):
                # tiny matmuls into a short-lived psum slot: keep the PE
                # activity monitor from re-throttling the clock during
                # ACT-bound micro-idles
                fill_i[0] += 1
                fps = pp.tile([128, QC], F32, tag="pp",
                              name=f"fill{fill_i[0]}")
                for _ in range(n):
                    nc.tensor.matmul(
                        fps[0:64, 0:64], lhsT=scratchB[:, 0:64],
                        rhs=scratchB[:, 0:64],
                        start=True, stop=True, skip_group_check=True,
                    )

            def emit_x0(q, t):
                if t == 0:
                    x0_tiles[q] = px0.tile([128, QC], F32, tag="x0",
                                           name=f"x0_{q}")
                    cur_x0[0] = x0_tiles[q]
                x0 = x0_tiles[q]
                eT = eT_tiles.pop((q, t))
                single = False   # col-packed chains for every qchunk
                for i in range(GRP):
                    kc = GRP * t + i
                    par = 0 if single else kc % 2
                    # each parity chain starts/stops its own partition
                    # range (pending-zero marking is per written partition)
                    nc.tensor.matmul(
                        x0[64 * par : 64 * par + CI + 1, :],
                        lhsT=hT_sb[:, kc, :],
                        rhs=eT[:, i, :],
                        start=kc <= (0 if single else 1),
                        stop=kc >= NKC - (1 if single else 2),
                        tile_position=(0, 64 * par),
                        skip_group_check=True,
                    )

            tail_state = {}

            def emit_tail_pre(q):
                # copy x0 out of PSUM (frees the accumulator bank early);
                # unnormalized channels + partial denominators, bf16
                x0 = x0_tiles.pop(q)
                if cur_x0[0] is x0:
                    cur_x0[0] = None
                x0s = smallp.tile([97, QC], BF16, tag="x0s")
                nc.vector.tensor_copy(x0s, x0[0:97, :])
                tail_state[q] = x0s

            def emit_tail_mid(q):
                x0s = tail_state[q]
                # d = dA + dB via a 1-column matmul (ones at rows 0, 64)
                dps = pp.tile([128, QC], F32, tag="pp", name=f"d{q}")
                nc.tensor.matmul(
                    dps[0:1, :], lhsT=dones, rhs=x0s,
                    start=True, stop=True,
                )
                rcp = smallp.tile([1, QC], F32, tag="rcp")
                nc.vector.reciprocal_approx_fast(out=rcp, in_=dps[0:1, :])
                rcp_b = smallp.tile([97, QC], F32, tag="rcpb")
                nc.gpsimd.partition_broadcast(rcp_b, rcp, channels=97)
                x0a = smallp.tile([97, QC], BF16, tag="x0a")
                nc.vector.tensor_mul(x0a, x0s, rcp_b)
                tail_state[q] = x0a

            def emit_tail_post(q):
                x0a = tail_state.pop(q)
                rows = 97
                for oc in (0, 1):
                    vps = pp.tile([128, QC], F32, tag="pp", name=f"v{q}_{oc}")
                    nc.tensor.matmul(
                        vps, lhsT=cb_sb[0:rows, 576 + 128 * oc : 704 + 128 * oc],
                        rhs=x0a[0:rows, :],
                        start=True, stop=True,
                    )
                    ot = outp.tile([128, QC], F32)
                    nc.vector.tensor_add(ot, vps, xbf_sb[:, oc, ts(q, QC)])
                    if q == NQC - 1:
                        for h in range(2):
                            nc.sync.dma_start(
                                out=outr[64 * h : 64 * h + 64, oc, ts(q, QC)],
                                in_=ot[64 * h : 64 * h + 64, :],
                            )
                    else:
                        nc.sync.dma_start(out=outr[:, oc, ts(q, QC)], in_=ot)

            # ---- emission schedule ----
            # qchunk 0 carries the pre-phase (f/g/hT) in its logits slots
            pre = {
                0: [lambda: emit_f(0), lambda: emit_g(0)],
                2: [lambda: emit_g(1)],
                4: [lambda: emit_g(2)],
                5: [lambda: emit_hT(0)],
                6: [lambda: emit_g(3)],
                7: [lambda: emit_f(1)],
                8: [lambda: emit_g(4)],
                9: [lambda: emit_hT(1)],
                10: [lambda: emit_g(5)],
                11: [lambda: emit_f(2)],
                12: [lambda: emit_g(6)],
                13: [lambda: emit_f(3)],
                14: [lambda: emit_g(7)],
            }
            seq = [(q, t) for q in range(NQC) for t in range(NGRP)]
            lg_i = 0
            x0_i = 0
            slot = 0
            deferred = []   # (due_slot, fn) in due order
            while lg_i < len(seq) or x0_i < len(seq) or deferred:
                if lg_i < len(seq):
                    pair = seq[lg_i : lg_i + 2]
                    for q, t in pair:
                        if q == 0:
                            for fn in pre.get(t, []):
                                fn()
                    emit_logits_pair(pair)
                    lg_i += len(pair)
                # x0 lags the exps (last qchunk drains eagerly instead)
                lag_req = 3
                budget = 2 if lg_i < len(seq) else len(seq)
                if lg_i - x0_i > 5:
                    budget = 3
                while (
                    budget > 0
                    and x0_i < len(seq)
                    and (lg_i - x0_i >= lag_req or lg_i >= len(seq))
                    and seq[x0_i] in eT_tiles
                ):
                    qx, tx = seq[x0_i]
                    # x0 must trail the hT emission (PE FIFO order):
                    # hT slot 0 lands at lg group 5, slot 1 at group 9
                    if qx == 0 and tx < 8 and lg_i < 6:
                        break
                    if qx == 0 and tx >= 8 and lg_i < 10:
                        break
                    emit_x0(qx, tx)
                    if lg_i >= len(seq):
                        emit_fillers(1)
                    x0_i += 1
                    budget -= 1
                    if tx == NGRP - 1:
                        emit_tail_pre(qx)
                        deferred.append(
                            (slot + 2, lambda q=qx: emit_tail_mid(q)))
                        deferred.append(
                            (slot + 3, lambda q=qx: emit_tail_post(q)))
                emit_fillers(2)
                while deferred and (
                    deferred[0][0] <= slot or
                    (lg_i >= len(seq) and x0_i >= len(seq))
                ):
                    deferred.pop(0)[1]()
                slot += 1

    nc.compile()
    return nc


def kernel(x, wf, bf, wg, bg, wh, bh, wv, bv, gamma):
    global _cached_nc, LAST_EXEC_NS
    if _cached_nc is None:
        _cached_nc = _build()
    nc = _cached_nc

    x = np.asarray(x, dtype=np.float32)
    wf = np.asarray(wf, dtype=np.float32)
    bf = np.asarray(bf, dtype=np.float32)
    wg = np.asarray(wg, dtype=np.float32)
    bg = np.asarray(bg, dtype=np.float32)
    wh = np.asarray(wh, dtype=np.float32)
    bh = np.asarray(bh, dtype=np.float32)
    wv = np.asarray(wv, dtype=np.float32)
    bv = np.asarray(bv, dtype=np.float32)
    g0 = float(np.asarray(gamma, dtype=np.float32).reshape(-1)[0])

    xf = np.ascontiguousarray(x.reshape(B, C, N))
    # f/g weights replicated 4x along M so f/g land replicated on the
    # four 32-partition strips (enables row-packed logits matmuls).
    bft = ml_dtypes.bfloat16
    wfT = np.tile(wf.T, (1, 4)).astype(np.float32)        # (256, 128)
    wgT = np.tile(wg.T, (1, 4)).astype(np.float32)
    whT = wh.T.astype(np.float32)                         # (256, 32)
    # stacked wv for the K=97 projection: bias rows at 0 and 64 (the
    # normalized denominators sum to 1), wv.T at rows 1-32 and 65-96.
    bias = g0 * (bv + wv @ bh)
    wvT = np.zeros((97, C), np.float32)
    wvT[0, :] = bias
    wvT[64, :] = bias
    wvT[1 : CI + 1, :] = g0 * wv.T
    wvT[65 : 65 + CI, :] = g0 * wv.T
    # single bf16 const blob: per-partition [wfT(2x128) wgT(2x128)
    # whT(2x32) wvT(2x128)]
    cb = np.zeros((128, 832), np.float32)
    cb[:, 0:128] = wfT[0:128, :]
    cb[:, 128:256] = wfT[128:256, :]
    cb[:, 256:384] = wgT[0:128, :]
    cb[:, 384:512] = wgT[128:256, :]
    cb[:, 512:544] = whT[0:128, :]
    cb[:, 544:576] = whT[128:256, :]
    cb[0:97, 576:832] = wvT
    cb = np.ascontiguousarray(cb.astype(bft))
    cf = np.ascontiguousarray(
        np.stack([np.tile(bf, 4), np.tile(bg, 4)], axis=1)
    ).astype(np.float32)

    in_maps = []
    for core in range(NCORES):
        b, half = divmod(core, 2)
        xb = xf[b]
        if half:
            xb = np.ascontiguousarray(
                np.concatenate([xb[:, NQ:], xb[:, :NQ]], axis=1)
            )
        xbb = xb.astype(bft)
        xbh = np.ascontiguousarray(
            np.concatenate([xbb[0:128, 0:512], xbb[128:256, 0:512]], axis=1)
        )
        in_maps.append({"xbf": xbb, "xbfh": xbh, "cb": cb, "cf": cf})

    res = run_bass_kernel_spmd(
        nc, in_maps, list(range(NCORES)),
        trace=TRACE or bool(os.environ.get("BASS_KERNEL_TRACE")),
    )
    LAST_EXEC_NS = res.exec_time_ns

    out = np.empty((B, C, N), np.float32)
    for core in range(NCORES):
        b, half = divmod(core, 2)
        out[b][:, half * NQ : (half + 1) * NQ] = res.results[core]["out"]
    return out.reshape(B, C, W, H)


# revision 40
# speedup vs baseline: 1.1089x; 1.0805x over previous
"""Trainium2 Bass kernel for the attention layer:

    f = wf@x+bf; g = wg@x+bg; h = wh@x+bh            (1x1 convs, Ci=32)
    attn = softmax(f^T g, axis=-1)                   (per batch, N=4096)
    out = (wv @ (h @ attn^T) + bv) * gamma + x

Sharding: 8 cores = 4 batches x 2 query-halves (2048 queries each).
Each core receives the full (256, 4096) batch slice with its query half
permuted to the front, so the SPMD program uses fixed offsets.

v2 dataflow (ACT-exp bound, everything else hides behind it):
  - all matmul operands bf16 (PSUM accumulate stays fp32); fp32 kept
    only for the residual add.
  - logits: key chunks of 128 in groups of 2 PSUM banks; 4-way
    strip-replicated f/g so consecutive chunk matmuls row-pack into
    different PE bands.  ACT exp reads the 2-bank group in ONE call
    (1024 elems/lane) -> bf16 eT in SBUF.
  - x0 accumulation col-packed 2-way: even key chunks at tile_position
    (0,0) (psum rows 0-32), odd at (0,64) (rows 64-96); each half
    carries its own ones-column for the softmax denominator.  One
    interleaved accumulation chain per qchunk bank (start at kc==0,
    stop at kc==31).
  - cross-qchunk software pipeline: the PE stream interleaves next
    qchunk's logits with current qchunk's x0 (x0 lags the exps by a
    few groups, bounded by the eT pool), so ACT never starves at
    qchunk boundaries and there are no serial per-qchunk tails.
  - tail per qchunk: dB moved to partition 0 (gpsimd), d=dA+dB,
    reciprocal_approx_fast, PE outer-product broadcast of 1/d to 97
    partitions, normalize (bf16), project with stacked wv (K=97, rows
    33-63 zero; bias rows at 0 and 64 exploit dA/d + dB/d = 1),
    fp32 residual add, DMA out.
"""

import os
import numpy as np
import ml_dtypes

import concourse.bass as bass
import concourse.mybir as mybir
import concourse.tile as tile
from concourse import bacc
from concourse.bass import ts
from concourse.bass_utils import run_bass_kernel_spmd

F32 = mybir.dt.float32
F32R = mybir.dt.float32r
BF16 = mybir.dt.bfloat16
EXP = mybir.ActivationFunctionType.Exp

B, C, W, H = 4, 256, 64, 64
N = W * H            # 4096 keys/queries per batch
CI = 32              # inner channels
NCORES = 8
NQ = N // 2          # queries per core
QC = 512             # query chunk = one fp32 PSUM bank
NQC = NQ // QC       # 4 query chunks per core
KC = 128             # key chunk = partition dim
NKC = N // KC        # 32 key chunks
GRP = 3              # key chunks per ACT exp group (PSUM banks per call)
NGRP = 11            # 10 groups of 3 + 1 of 2 per qchunk
NWARM = 24           # dummy matmuls to warm the PE clock gate

TRACE = False
LAST_EXEC_NS = None

_cached_nc = None


def _build():
    nc = bacc.Bacc(
        "TRN2", target_bir_lowering=False, debug=False, num_devices=NCORES
    )
    xbf_d = nc.dram_tensor("xbf", (C, N), BF16, kind="ExternalInput").ap()
    xbfh_d = nc.dram_tensor("xbfh", (128, 1024), BF16, kind="ExternalInput").ap()
    cb_d = nc.dram_tensor("cb", (128, 832), BF16, kind="ExternalInput").ap()
    cf_d = nc.dram_tensor("cf", (128, 2), F32, kind="ExternalInput").ap()
    out_d = nc.dram_tensor("out", (C, NQ), F32, kind="ExternalOutput").ap()

    xbfr = xbf_d.rearrange("(cc p) n -> p cc n", p=128)
    outr = out_d.rearrange("(oc p) n -> p oc n", p=128)

    with tile.TileContext(nc) as tc:
        with (
            tc.tile_pool(name="consts", bufs=1) as consts,
            tc.tile_pool(name="data", bufs=1) as data,
            tc.tile_pool(name="eTp", bufs=8) as eTp,
            tc.tile_pool(name="smallp", bufs=2) as smallp,
            tc.tile_pool(name="outp", bufs=3) as outp,
            tc.tile_pool(name="pl", bufs=2, space="PSUM") as pl,
            tc.tile_pool(name="pp", bufs=1, space="PSUM") as pp,
            tc.tile_pool(name="px0", bufs=1, space="PSUM") as px0,
        ):
            # ---- PE + ACT warm-up (overlaps the input DMAs) ----
            scratch = consts.tile([128, QC], F32)
            nc.vector.memset(scratch, 0.0)
            wps = pp.tile([128, QC], F32, tag="pp", name="warm")
            for i in range(NWARM):
                # small N so the warm-ups don't delay the real stream
                nc.tensor.matmul(
                    wps[:, 0:128], lhsT=scratch[:, 0:128],
                    rhs=scratch[:, 0:128],
                    start=True, stop=True, skip_group_check=True,
                )
            scratchB = consts.tile([128, 64], BF16)
            nc.vector.memset(scratchB, 0.0)
            scratch2 = consts.tile([1, 8], F32)
            nc.scalar.activation(out=scratch2, in_=scratch[0:1, 0:8], func=EXP)

            # ---- x (bf16 everywhere, also used for the residual) ----
            # dma_start triggers cost ~640 ns SERIAL on the Sync engine:
            # keep the transfer count minimal.  One flat head transfer
            # (cols 0-511, partition-contiguous host layout) unblocks
            # f0/g0; the rest arrives in 4 column stripes.
            xbf_sb = data.tile([128, 2, N], BF16)
            nc.sync.dma_start(
                out=xbf_sb[:, :, 0:512],
                in_=xbfh_d.rearrange("p (cc n) -> p cc n", cc=2),
            )
            # ---- constants: one bf16 blob + one f32 blob ----
            cb_sb = consts.tile([128, 832], BF16)
            nc.sync.dma_start(out=cb_sb, in_=cb_d)
            cf_sb = consts.tile([128, 2], F32)
            nc.sync.dma_start(out=cf_sb, in_=cf_d)
            bf_sb = cf_sb[:, 0:1]
            bg_sb = cf_sb[:, 1:2]
            # rest of xbf (columns 512-4095) in 4 stripes
            for c0, c1 in ((512, 1024), (1024, 2048), (2048, 3072),
                           (3072, 4096)):
                nc.sync.dma_start(
                    out=xbf_sb[:, :, c0:c1], in_=xbfr[:, :, c0:c1]
                )
            # ones at rows 0 and 64: the denominator-extractor stationary
            dones = consts.tile([97, 1], BF16)
            nc.vector.memset(dones, 0.0)
            nc.vector.memset(dones[0:1, :], 1.0)
            nc.vector.memset(dones[64:65, :], 1.0)

            f_sb = data.tile([128, NQ], BF16)
            g_sb = data.tile([128, N], BF16)
            hT_sb = data.tile([128, NKC, CI + 1], BF16)
            nc.vector.memset(hT_sb[:, :, 0:1], 1.0)

            # ---- pre-phase emitters (short-lived psum slots, pp pool) ----
            def emit_f(j, pool, tag):
                ps = pool.tile([128, QC], F32, tag=tag, name=f"psf{j}")
                for cc in range(2):
                    nc.tensor.matmul(
                        ps, lhsT=cb_sb[:, ts(cc, 128)],
                        rhs=xbf_sb[:, cc, ts(j, QC)],
                        start=cc == 0, stop=cc == 1,
                    )
                nc.vector.tensor_scalar_add(f_sb[:, ts(j, QC)], ps, bf_sb)

            def emit_g(j, pool, tag):
                ps = pool.tile([128, QC], F32, tag=tag, name=f"psg{j}")
                for cc in range(2):
                    nc.tensor.matmul(
                        ps, lhsT=cb_sb[:, 256 + 128 * cc : 384 + 128 * cc],
                        rhs=xbf_sb[:, cc, ts(j, QC)],
                        start=cc == 0, stop=cc == 1,
                    )
                nc.vector.tensor_scalar_add(g_sb[:, ts(j, QC)], ps, bg_sb)

            def emit_hT(slot, pool, tag):
                # 16 key chunks' hT per [128, 512] psum slot, one big
                # strided copy into the ones-augmented hT_sb layout
                ps = pool.tile([128, 16, 32], F32, tag=tag, name=f"psh{slot}")
                for m in range(16):
                    kc = 16 * slot + m
                    for cc in range(2):
                        nc.tensor.matmul(
                            ps[:, m, :],
                            lhsT=xbf_sb[:, cc, ts(kc, KC)],
                            rhs=cb_sb[:, 512 + 32 * cc : 544 + 32 * cc],
                            start=cc == 0, stop=cc == 1,
                            skip_group_check=True,
                        )
                nc.vector.tensor_copy(
                    hT_sb[:, 16 * slot : 16 * slot + 16, 1 : CI + 1], ps
                )

            # ---- main-loop emitters ----
            eT_tiles = {}

            def grp_chunks(t):
                return list(range(3 * t, min(3 * t + 3, NKC)))

            def emit_logits(q, t):
                kcs = grp_chunks(t)
                w = len(kcs)
                ps = pl.tile([128, GRP, QC], F32, tag="lg",
                             name=f"lg{q}_{t}")
                for i, kc in enumerate(kcs):
                    s = kc % 4
                    nc.tensor.matmul(
                        ps[:, i, :],
                        lhsT=g_sb[32 * s : 32 * s + 32, ts(kc, KC)],
                        rhs=f_sb[32 * s : 32 * s + 32, ts(q, QC)],
                        start=True, stop=True,
                        tile_position=(32 * s, 0),
                    )
                eT = eTp.tile([128, GRP, QC], BF16, tag="eT")
                nc.scalar.activation(
                    out=eT[:, 0:w, :], in_=ps[:, 0:w, :], func=EXP
                )
                eT_tiles[(q, t)] = eT

            x0_tiles = {}
            cur_x0 = [None]

            fill_i = [0]

            def emit_fillers(n=2):
                # tiny matmuls into a short-lived psum slot: keep the PE
                # activity monitor from re-throttling the clock during
                # ACT-bound micro-idles
                fill_i[0] += 1
                fps = pp.tile([128, QC], F32, tag="pp",
                              name=f"fill{fill_i[0]}")
                for _ in range(n):
                    nc.tensor.matmul(
                        fps[0:64, 0:64], lhsT=scratchB[:, 0:64],
                        rhs=scratchB[:, 0:64],
                        start=True, stop=True, skip_group_check=True,
                    )

            def emit_x0(q, t):
                if t == 0:
                    x0_tiles[q] = px0.tile([128, QC], F32, tag="x0",
                                           name=f"x0_{q}")
                    cur_x0[0] = x0_tiles[q]
                x0 = x0_tiles[q]
                eT = eT_tiles.pop((q, t))
                for i, kc in enumerate(grp_chunks(t)):
                    par = kc % 2
                    # each parity chain starts/stops its own partition
                    # range (pending-zero marking is per written partition)
                    nc.tensor.matmul(
                        x0[64 * par : 64 * par + CI + 1, :],
                        lhsT=hT_sb[:, kc, :],
                        rhs=eT[:, i, :],
                        start=kc <= 1,
                        stop=kc >= NKC - 2,
                        tile_position=(0, 64 * par),
                        skip_group_check=True,
                    )

            tail_state = {}

            def emit_tail_pre(q):
                # copy x0 out of PSUM (frees the accumulator bank early);
                # unnormalized channels + partial denominators, bf16
                x0 = x0_tiles.pop(q)
                if cur_x0[0] is x0:
                    cur_x0[0] = None
                x0s = smallp.tile([97, QC], BF16, tag="x0s")
                nc.vector.tensor_copy(x0s, x0[0:97, :])
                tail_state[q] = x0s

            def emit_tail_mid(q):
                x0s = tail_state[q]
                # d = dA + dB via a 1-column matmul (ones at rows 0, 64)
                dps = pp.tile([128, QC], F32, tag="pp", name=f"d{q}")
                nc.tensor.matmul(
                    dps[0:1, :], lhsT=dones, rhs=x0s,
                    start=True, stop=True,
                )
                rcp = smallp.tile([1, QC], F32, tag="rcp")
                nc.vector.reciprocal_approx_fast(out=rcp, in_=dps[0:1, :])
                rcp_b = smallp.tile([97, QC], F32, tag="rcpb")
                nc.gpsimd.partition_broadcast(rcp_b, rcp, channels=97)
                x0a = smallp.tile([97, QC], BF16, tag="x0a")
                nc.vector.tensor_mul(x0a, x0s, rcp_b)
                tail_state[q] = x0a

            def emit_tail_post(q):
                x0a = tail_state.pop(q)
                rows = 97
                for oc in (0, 1):
                    vps = pp.tile([128, QC], F32, tag="pp", name=f"v{q}_{oc}")
                    nc.tensor.matmul(
                        vps, lhsT=cb_sb[0:rows, 576 + 128 * oc : 704 + 128 * oc],
                        rhs=x0a[0:rows, :],
                        start=True, stop=True,
                    )
                    ot = outp.tile([128, QC], F32)
                    nc.vector.tensor_add(ot, vps, xbf_sb[:, oc, ts(q, QC)])
                    nc.sync.dma_start(out=outr[:, oc, ts(q, QC)], in_=ot)

            # ---- emission schedule ----
            # qchunk 0 carries the whole pre-phase (f/g/hT) in its first
            # six logits slots, alternating between the pp and px0 banks
            # (x0 accumulation starts only after the pre-phase is done)
            pre_items = [
                lambda p, g: emit_f(0, p, g), lambda p, g: emit_g(0, p, g),
                lambda p, g: emit_g(1, p, g), lambda p, g: emit_hT(0, p, g),
                lambda p, g: emit_g(2, p, g), lambda p, g: emit_f(1, p, g),
                lambda p, g: emit_g(3, p, g), lambda p, g: emit_g(4, p, g),
                lambda p, g: emit_f(2, p, g), lambda p, g: emit_g(5, p, g),
                lambda p, g: emit_hT(1, p, g), lambda p, g: emit_g(6, p, g),
                lambda p, g: emit_f(3, p, g), lambda p, g: emit_g(7, p, g),
            ]
            pre = {
                0: pre_items[0:2],
                1: pre_items[2:4],
                2: pre_items[4:7],
                3: pre_items[7:9],
                4: pre_items[9:11],
                5: pre_items[11:14],
            }
            pre_tgl = [0]
            seq = [(q, t) for q in range(NQC) for t in range(NGRP)]
            lg_i = 0
            x0_i = 0
            slot = 0
            deferred = []   # (due_slot, fn) in due order
            pools = [(pp, "pp"), (px0, "x0")]
            while lg_i < len(seq) or x0_i < len(seq) or deferred:
                if lg_i < len(seq):
                    q, t = seq[lg_i]
                    if q == 0:
                        for fn in pre.get(t, []):
                            p, g = pools[pre_tgl[0] % 2]
                            pre_tgl[0] += 1
                            fn(p, g)
                    emit_logits(q, t)
                    lg_i += 1
                # x0 lags the exps; starts only after the full pre-phase
                lag_req = 2
                budget = 2 if lg_i < len(seq) else len(seq)
                if lg_i - x0_i > 4:
                    budget = 3
                while (
                    budget > 0
                    and x0_i < len(seq)
                    and (lg_i - x0_i >= lag_req or lg_i >= len(seq))
                    and seq[x0_i] in eT_tiles
                    and lg_i >= 6
                ):
                    qx, tx = seq[x0_i]
                    emit_x0(qx, tx)
                    if lg_i >= len(seq):
                        emit_fillers(1)
                    x0_i += 1
                    budget -= 1
                    if tx == NGRP - 1:
                        emit_tail_pre(qx)
                        deferred.append(
                            (slot + 2, lambda q=qx: emit_tail_mid(q)))
                        deferred.append(
                            (slot + 3, lambda q=qx: emit_tail_post(q)))
                fired = False
                while deferred and (
                    deferred[0][0] <= slot or
                    (lg_i >= len(seq) and x0_i >= len(seq))
                ):
                    deferred.pop(0)[1]()
                    fired = True
                if not fired:
                    emit_fillers(2)
                slot += 1

    nc.compile()
    return nc


def kernel(x, wf, bf, wg, bg, wh, bh, wv, bv, gamma):
    global _cached_nc, LAST_EXEC_NS
    if _cached_nc is None:
        _cached_nc = _build()
    nc = _cached_nc

    x = np.asarray(x, dtype=np.float32)
    wf = np.asarray(wf, dtype=np.float32)
    bf = np.asarray(bf, dtype=np.float32)
    wg = np.asarray(wg, dtype=np.float32)
    bg = np.asarray(bg, dtype=np.float32)
    wh = np.asarray(wh, dtype=np.float32)
    bh = np.asarray(bh, dtype=np.float32)
    wv = np.asarray(wv, dtype=np.float32)
    bv = np.asarray(bv, dtype=np.float32)
    g0 = float(np.asarray(gamma, dtype=np.float32).reshape(-1)[0])

    xf = np.ascontiguousarray(x.reshape(B, C, N))
    # f/g weights replicated 4x along M so f/g land replicated on the
    # four 32-partition strips (enables row-packed logits matmuls).
    bft = ml_dtypes.bfloat16
    wfT = np.tile(wf.T, (1, 4)).astype(np.float32)        # (256, 128)
    wgT = np.tile(wg.T, (1, 4)).astype(np.float32)
    whT = wh.T.astype(np.float32)                         # (256, 32)
    # stacked wv for the K=97 projection: bias rows at 0 and 64 (the
    # normalized denominators sum to 1), wv.T at rows 1-32 and 65-96.
    bias = g0 * (bv + wv @ bh)
    wvT = np.zeros((97, C), np.float32)
    wvT[0, :] = bias
    wvT[64, :] = bias
    wvT[1 : CI + 1, :] = g0 * wv.T
    wvT[65 : 65 + CI, :] = g0 * wv.T
    # single bf16 const blob: per-partition [wfT(2x128) wgT(2x128)
    # whT(2x32) wvT(2x128)]
    cb = np.zeros((128, 832), np.float32)
    cb[:, 0:128] = wfT[0:128, :]
    cb[:, 128:256] = wfT[128:256, :]
    cb[:, 256:384] = wgT[0:128, :]
    cb[:, 384:512] = wgT[128:256, :]
    cb[:, 512:544] = whT[0:128, :]
    cb[:, 544:576] = whT[128:256, :]
    cb[0:97, 576:832] = wvT
    cb = np.ascontiguousarray(cb.astype(bft))
    cf = np.ascontiguousarray(
        np.stack([np.tile(bf, 4), np.tile(bg, 4)], axis=1)
    ).astype(np.float32)

    in_maps = []
    for core in range(NCORES):
        b, half = divmod(core, 2)
        xb = xf[b]
        if half:
            xb = np.ascontiguousarray(
                np.concatenate([xb[:, NQ:], xb[:, :NQ]], axis=1)
            )
        xbb = xb.astype(bft)
        xbh = np.ascontiguousarray(
            np.concatenate([xbb[0:128, 0:512], xbb[128:256, 0:512]], axis=1)
        )
        in_maps.append({"xbf": xbb, "xbfh": xbh, "cb": cb, "cf": cf})

    res = run_bass_kernel_spmd(
        nc, in_maps, list(range(NCORES)),
        trace=TRACE or bool(os.environ.get("BASS_KERNEL_TRACE")),
    )
    LAST_EXEC_NS = res.exec_time_ns

    out = np.empty((B, C, N), np.float32)
    for core in range(NCORES):
        b, half = divmod(core, 2)
        out[b][:, half * NQ : (half + 1) * NQ] = res.results[core]["out"]
    return out.reshape(B, C, W, H)
